# revision 27
# baseline (speedup 1.0000x reference)
"""Bayesian curve filter kernel for Trainium2 (8 NeuronCores, SPMD).

Sharding: data-parallel over the 1024 Monte-Carlo samples -> 128 per core
(exactly the SBUF partition count; samples live on partitions).

Device algorithm per core (all fp32):
  1. out1 = curves^T @ R : per-sample curve points / velocity / accel
     [128s, 180] (cols 0-59 pts, 60-119 v_t, 120-179 a_t) per dim d.
  2. speeds / centripetal / braking-interp pipeline on [128, 60] tiles.
  3. Boundary nearest-neighbor (the heavy part), soft-select formulation:
       s1[s,b]   = 2 x.b - |b|^2            (argmax_b s1 == argmin_b d2)
       m[s]      = max_b s1                 (DVE reduce over 2x1000 scores)
       t[b,s]    = m - s1 >= 0              (PE re-emission, [b,s] layout)
       H         = exp(-K t)                (ACT; ~one-hot at the argmin)
       sel[4,s]  = sum_b H_b * (e_b, cnx_b, cny_b, 1)   (PE contraction)
       dist      = (sel0 - px*sel1 - py*sel2) / sel3
  4. Per-sample log-score -> w; partial (sum_s w*curve_s, sum_s w) via a
     final [128,17]x[128,1] matmul -> [17] per core; host sums across the
     8 cores and divides (softmax normalization cancels globally).
"""

import os
import numpy as np
from math import comb

import concourse.bass as bass
import concourse.bacc as bacc
import concourse.mybir as mybir
from concourse import tile
from concourse import bass_utils

F32 = mybir.dt.float32
F32R = mybir.dt.float32r
BF16 = mybir.dt.bfloat16
F16 = mybir.dt.float16
U32 = mybir.dt.uint32
ALU = mybir.AluOpType
AF = mybir.ActivationFunctionType
AX = mybir.AxisListType


def _r(ap):
    return ap.bitcast(F32R)

NCORES = 8
S_FULL = 1024
SC = 128          # samples per core
P = 60            # points per curve
NB = 1000         # boundary points per boundary
NBP = 1024        # padded
ORD = 7           # bezier order
BETA_SPEED = 0.1
MAX_CA = 19.6
NSEG = 19         # interp segments (20 knots)
NCC = 128         # coarse centers per boundary (level-1 max estimate)
NQ_C = 15         # quads

_cache = {}


def _diff_mat(n):
    # D [n, n+1]: (D @ c)[k] = c[k+1] - c[k]
    D = np.zeros((n, n + 1), np.float64)
    for k in range(n):
        D[k, k] = -1.0
        D[k, k + 1] = 1.0
    return D


def _build_program(interp_x, interp_dx, interp_m, y0):
    """Builds the bass program. interp constants are baked as immediates."""
    nc = bacc.Bacc("TRN2", target_bir_lowering=False, debug=False, enable_asserts=False)

    # ---- DRAM I/O ----
    d_cv = nc.dram_tensor("cv", [16, SC], F32, kind="ExternalInput").ap()       # curvesT: rows 0-7 x-coefs, 8-15 y
    d_cf = nc.dram_tensor("cf17", [SC, 17], F32, kind="ExternalInput").ap()     # curves flat + ones col
    d_R = nc.dram_tensor("Rm", [8, 180], F32, kind="ExternalInput").ap()
    d_bG = nc.dram_tensor("bG", [6, 2 * NBP], F32R, kind="ExternalInput").ap()   # em2 lhsT [-2bx;-2by;b2Chi;b2Clo;1;1]
    d_tb = nc.dram_tensor("tb", [SC, 512], BF16, kind="ExternalInput").ap()       # select lhsT chunks [ehi,elo,nxhi,nxlo,nyhi,nylo,1,0]
    d_Th = nc.dram_tensor("Th", [SC, 1], F32, kind="ExternalInput").ap()  # is_le threshold
    d_I8 = nc.dram_tensor("I8", [8, 8], F32, kind="ExternalInput").ap()
    d_I128 = nc.dram_tensor("I128", [SC, SC], F32, kind="ExternalInput").ap()
    d_Kv = nc.dram_tensor("Kv", [SC, 1], F32, kind="ExternalInput").ap()  # -K replicated
    d_ones = nc.dram_tensor("ones_row", [1, P * SC], F32R, kind="ExternalInput").ap()
    d_cfT = nc.dram_tensor("cfT", [18, SC], F16, kind="ExternalInput").ap()     # [X8; Y8; 1; 1] per sample
    d_R2c = nc.dram_tensor("R2c", [18, NQ_C * 1024], F16, kind="ExternalInput").ap()  # center-score table
    d_out = nc.dram_tensor("out17", [17, 1], F32, kind="ExternalOutput").ap()
    d_diag = nc.dram_tensor("diag", [SC, 8], F32, kind="ExternalOutput").ap()

    NQ = 15  # quads of p (4 p's each -> 512-wide sp blocks)

    with tile.TileContext(nc) as tc:
        with (
            tc.tile_pool(name="cst", bufs=1) as cst,
            tc.tile_pool(name="paug", bufs=1) as paugp,
            tc.tile_pool(name="selc", bufs=1) as selcp,
            tc.tile_pool(name="selc2", bufs=3) as selcp2,
            tc.tile_pool(name="hbuf", bufs=4) as hbuf,
            tc.tile_pool(name="wk", bufs=4) as wk,
            tc.tile_pool(name="m2", bufs=4) as m2p,
            tc.tile_pool(name="big", bufs=2, space="PSUM") as big,
            tc.tile_pool(name="sml", bufs=3, space="PSUM") as sml,
        ):
            # ---- load constants (replicated at partition bases 0/32/64/96) ----
            cvx = cst.tile([8, SC], F32)
            nc.sync.dma_start(cvx[:], d_cv[0:8, :])
            cvy = cst.tile([8, SC], F32)
            nc.sync.dma_start(cvy[:], d_cv[8:16, :])
            cf = cst.tile([SC, 17], F32)
            nc.sync.dma_start(cf[:], d_cf)
            Rm = cst.tile([8, 180], F32)
            nc.sync.dma_start(Rm[:], d_R)
            cfT = cst.tile([18, SC], F16)
            nc.sync.dma_start(cfT[:], d_cfT)
            R2c = cst.tile([18, NQ_C * 1024], F16)
            nc.sync.dma_start(R2c[:], d_R2c)
            I128 = cst.tile([SC, SC], F32)
            nc.sync.dma_start(I128[:], d_I128)
            bG = cst.tile([102, 2 * NBP], F32R)
            I8r = cst.tile([8, 8], F32)
            nc.scalar.dma_start(I8r[:], d_I8)
            for j in range(4):
                nc.scalar.dma_start(bG[32 * j:32 * j + 6, :], d_bG)
            tb = cst.tile([SC, 512], BF16)
            thv = cst.tile([SC, 1], F32)
            nc.scalar.dma_start(thv[:], d_Th)
            b25 = cst.tile([SC, 1], F32)
            nc.vector.memset(b25[:], -25.0)
            nc.scalar.dma_start(tb[:], d_tb)
            Kv = cst.tile([SC, 1], F32)
            nc.scalar.dma_start(Kv[:], d_Kv)

            # ---- pts/vel/accel in [s, col] layout ----
            o1x = sml.tile([SC, 180], F32, tag="sm")
            nc.tensor.matmul(o1x[:], cvx[:], Rm[:], start=True, stop=True)
            o1y = sml.tile([SC, 180], F32, tag="sm")
            nc.tensor.matmul(o1y[:], cvy[:], Rm[:], start=True, stop=True)
            ox = cst.tile([SC, 180], F32)
            nc.vector.tensor_copy(ox[:], o1x[:])
            oy = cst.tile([SC, 180], F32)
            nc.vector.tensor_copy(oy[:], o1y[:])

            # ---- pts in [p, s] layout -> paug rows ----
            ptx = sml.tile([P, SC], F32, tag="sm")
            nc.tensor.matmul(ptx[:], Rm[:, 0:P], cvx[:], start=True, stop=True)
            pty = sml.tile([P, SC], F32, tag="sm")
            nc.tensor.matmul(pty[:], Rm[:, 0:P], cvy[:], start=True, stop=True)
            ptxs = cst.tile([P, SC], F32)
            nc.vector.tensor_copy(ptxs[:], ptx[:])
            ptys = cst.tile([P, SC], F32)
            nc.vector.tensor_copy(ptys[:], pty[:])

            pgi = paugp.tile([102, P * SC], F32R)  # rows 32j..+5: [px; py; 1; 1; mhi_in; mlo_in]
            pgo = paugp.tile([102, P * SC], F32R)  # rows 32j..+5: [px; py; 1; 1; mhi_out; mlo_out]
            qs = [nc.sync, nc.scalar, nc.gpsimd]
            di = 0
            for j in range(4):
                for pg in (pgi, pgo):
                    qs[di % 3].dma_start(pg[32 * j:32 * j + 1, :].rearrange("o (p s) -> o p s", p=P), _r(ptxs[:])); di += 1
                    qs[di % 3].dma_start(pg[32 * j + 1:32 * j + 2, :].rearrange("o (p s) -> o p s", p=P), _r(ptys[:])); di += 1
                    qs[di % 3].dma_start(pg[32 * j + 2:32 * j + 3, :], d_ones); di += 1
                    qs[di % 3].dma_start(pg[32 * j + 3:32 * j + 4, :], d_ones); di += 1

            # ---- boundary: per-quad pipeline, 4-way row/col tiled matmuls ----
            dTs = selcp.tile([SC, NQ * 64], F32)
            m2qs = [None] * NQ

            def em1(q):
                m2q = m2p.tile([SC, 64], F32, tag="m2")  # 4x copies: mhi blk 0:32, mlo blk 32:64
                m2qs[q] = m2q
                # coarse center scores: [s, (p4, bd2, c128)] via K=18 matmul
                for hh in range(2):
                    cs = sml.tile([SC, 512], F32, tag="sm")
                    nc.tensor.matmul(cs[:], cfT[:],
                                     R2c[:, q * 1024 + hh * 512: q * 1024 + (hh + 1) * 512],
                                     start=True, stop=True)
                    nc.vector.tensor_reduce(
                        m2q[:, 32 + 4 * hh: 36 + 4 * hh],
                        cs[:].rearrange("s (g c) -> s g c", c=NCC),
                        axis=AX.X, op=ALU.max)
                # mhi = trunc22(m) -> cols 0:8; mlo = trunc22(m - mhi) -> cols 32:40
                nc.vector.tensor_scalar(
                    m2q[:, 0:8].bitcast(U32), m2q[:, 32:40].bitcast(U32),
                    0xFFFFF000, None, op0=ALU.bitwise_and)
                nc.vector.tensor_sub(m2q[:, 32:40], m2q[:, 32:40], m2q[:, 0:8])
                nc.vector.tensor_scalar(
                    m2q[:, 32:40].bitcast(U32), m2q[:, 32:40].bitcast(U32),
                    0xFFFFF000, None, op0=ALU.bitwise_and)
                # replicate each 8-vec 3 more times within its 32-block
                nc.vector.tensor_copy(
                    m2q[:, 8:32].rearrange("s (r v) -> s r v", v=8),
                    m2q[:, 0:8].rearrange("s (r v) -> s r v", r=1).to_broadcast((SC, 3, 8)))
                nc.vector.tensor_copy(
                    m2q[:, 40:64].rearrange("s (r v) -> s r v", v=8),
                    m2q[:, 32:40].rearrange("s (r v) -> s r v", r=1).to_broadcast((SC, 3, 8)))
                mT = sml.tile([64, SC], F32, tag="sm")
                nc.tensor.matmul(mT[:], m2q[:], I128[:], start=True, stop=True)
                mTs = wk.tile([64, SC], F32, tag="mts")
                nc.scalar.copy(mTs[:], mT[:])
                qc = slice(q * 512, (q + 1) * 512)
                nc.sync.dma_start(
                    pgi[4:102:32, qc].rearrange("g (j2 s) -> g j2 s", j2=4), _r(mTs[0:32:2, :]))
                nc.scalar.dma_start(
                    pgo[4:102:32, qc].rearrange("g (j2 s) -> g j2 s", j2=4), _r(mTs[1:32:2, :]))
                nc.sync.dma_start(
                    pgi[5:102:32, qc].rearrange("g (j2 s) -> g j2 s", j2=4), _r(mTs[32:64:2, :]))
                nc.scalar.dma_start(
                    pgo[5:102:32, qc].rearrange("g (j2 s) -> g j2 s", j2=4), _r(mTs[33:64:2, :]))

            def em2(q):
                selc = selcp2.tile([8, 1024], F32, tag="selc")
                for bd in range(2):
                    pg = pgi if bd == 0 else pgo
                    sp = sml.tile([8, 512], F32, tag="sm")
                    for hw in range(4):  # windows of 2 chunks
                        t2 = big.tile([SC, NBP], F32, tag="big")
                        for cc in range(2):
                            c = 2 * hw + cc
                            g = 32 * (c % 4)
                            nc.tensor.matmul(
                                t2[:, cc * 512:(cc + 1) * 512],
                                bG[g:g + 6, bd * NBP + c * SC: bd * NBP + (c + 1) * SC],
                                pg[g:g + 6, q * 512:(q + 1) * 512],
                                start=True, stop=True, tile_position=(g, 0))
                        Ht = hbuf.tile([SC, NBP], BF16, tag="h")
                        if bd == 0:
                            nc.scalar.activation(Ht[:], t2[:], AF.Exp, scale=Kv[:], bias=b25[:])
                        else:
                            nc.vector.tensor_scalar(Ht[:], t2[:], thv[:], None, op0=ALU.is_le)
                        for cc in range(2):
                            c = 2 * hw + cc
                            nc.tensor.matmul(
                                sp[:], tb[:, (bd * 8 + c) * 32:(bd * 8 + c) * 32 + 8],
                                Ht[:, cc * 512:(cc + 1) * 512],
                                start=(c == 0), stop=(c == 7))
                    nc.scalar.copy(selc[:, bd * 512:(bd + 1) * 512], sp[:])
                dTq = sml.tile([SC, 64], F32, tag="sm")
                for j4 in range(4):
                    for bd in range(2):
                        off = bd * 512 + j4 * SC
                        nc.tensor.matmul(
                            dTq[:, j4 * 16 + bd * 8: j4 * 16 + (bd + 1) * 8],
                            selc[:, off: off + SC], I8r[:],
                            start=True, stop=True)
                nc.scalar.copy(dTs[:, q * 64:(q + 1) * 64], dTq[:])

            for q in range(NQ + 3):
                if q < NQ:
                    em1(q)
                if q >= 3:
                    em2(q - 3)

            # ---- speeds / accel pipeline [128, 60] ----
            vx, vy, ax_, ay = (ox[:, 60:120], oy[:, 60:120], ox[:, 120:180], oy[:, 120:180])
            spd2 = wk.tile([SC, P], F32)
            nc.vector.tensor_mul(spd2[:], vx, vx)
            t0 = wk.tile([SC, P], F32)
            nc.vector.tensor_mul(t0[:], vy, vy)
            nc.vector.tensor_add(spd2[:], spd2[:], t0[:])
            spd = wk.tile([SC, P], F32)
            nc.scalar.activation(spd[:], spd2[:], AF.Sqrt)
            rspd = wk.tile([SC, P], F32)
            nc.vector.reciprocal(rspd[:], spd[:])
            adv = wk.tile([SC, P], F32)
            nc.vector.tensor_mul(adv[:], ax_, vx)
            nc.vector.tensor_mul(t0[:], ay, vy)
            nc.vector.tensor_add(adv[:], adv[:], t0[:])
            lin = wk.tile([SC, P], F32)
            nc.vector.tensor_mul(lin[:], adv[:], rspd[:])
            a2 = wk.tile([SC, P], F32)
            nc.vector.tensor_mul(a2[:], ax_, ax_)
            nc.vector.tensor_mul(t0[:], ay, ay)
            nc.vector.tensor_add(a2[:], a2[:], t0[:])
            nc.vector.tensor_mul(t0[:], lin[:], lin[:])
            nc.vector.tensor_sub(a2[:], a2[:], t0[:])  # ca^2 (may be ~-eps)
            camax2 = wk.tile([SC, 1], F32)
            nc.vector.tensor_reduce(camax2[:], a2[:], axis=AX.X, op=ALU.max)
            nc.vector.tensor_scalar_max(camax2[:], camax2[:], 0.0)
            camax = wk.tile([SC, 1], F32)
            nc.scalar.activation(camax[:], camax2[:], AF.Sqrt)

            avg = wk.tile([SC, 1], F32)
            nc.vector.tensor_reduce(avg[:], spd[:], axis=AX.X, op=ALU.add)

            # braking interp: bl = y0 + sum_i m_i * clip(spd - x_i, 0, dx_i)
            bl = wk.tile([SC, P], F32)
            nc.vector.memset(bl[:], float(y0))
            ti = wk.tile([SC, P], F32)
            for i in range(NSEG):
                nc.vector.tensor_scalar(ti[:], spd[:], float(interp_x[i]), 0.0, op0=ALU.subtract, op1=ALU.max)
                nc.vector.tensor_scalar(ti[:], ti[:], float(interp_dx[i]), float(interp_m[i]), op0=ALU.min, op1=ALU.mult)
                nc.vector.tensor_add(bl[:], bl[:], ti[:])
            bv = wk.tile([SC, P], F32)
            nc.vector.tensor_sub(bv[:], lin[:], bl[:])
            worst = wk.tile([SC, 1], F32)
            nc.vector.tensor_reduce(worst[:], bv[:], axis=AX.X, op=ALU.min)
            nc.vector.tensor_scalar_min(worst[:], worst[:], 0.0)

            # ---- phase C: dist + maxes (transposes done per-quad above) ----
            bmax = wk.tile([SC, 1], F32)
            for bd in range(2):
                Se = wk.tile([SC, P], F32, tag="c0")
                nc.vector.tensor_add(Se[:], dTs[:, bd * 8 + 0:960:16], dTs[:, bd * 8 + 1:960:16])
                Scx = wk.tile([SC, P], F32, tag="c1")
                nc.vector.tensor_add(Scx[:], dTs[:, bd * 8 + 2:960:16], dTs[:, bd * 8 + 3:960:16])
                Scy = wk.tile([SC, P], F32, tag="c2")
                nc.vector.tensor_add(Scy[:], dTs[:, bd * 8 + 4:960:16], dTs[:, bd * 8 + 5:960:16])
                Sn = dTs[:, bd * 8 + 6:960:16]
                Se = Se[:]
                Scx = Scx[:]
                Scy = Scy[:]
                n1 = wk.tile([SC, P], F32, tag="d1")
                nc.vector.tensor_mul(n1[:], ox[:, 0:P], Scx)
                n2 = wk.tile([SC, P], F32, tag="d2")
                nc.vector.tensor_mul(n2[:], oy[:, 0:P], Scy)
                nc.vector.tensor_sub(n1[:], Se, n1[:])
                nc.vector.tensor_sub(n1[:], n1[:], n2[:])
                rs = wk.tile([SC, P], F32, tag="d3")
                nc.vector.reciprocal(rs[:], Sn)
                nc.vector.tensor_mul(n1[:], n1[:], rs[:])
                dm = wk.tile([SC, 1], F32, tag="d4")
                nc.vector.tensor_reduce(dm[:], n1[:], axis=AX.X, op=ALU.max)
                if bd == 0:
                    nc.vector.tensor_copy(bmax[:], dm[:])
                else:
                    nc.vector.tensor_max(bmax[:], bmax[:], dm[:])
            nc.vector.tensor_scalar_max(bmax[:], bmax[:], 0.0)

            # ---- per-sample scores -> w ----
            args = wk.tile([SC, 1], F32)
            nc.vector.tensor_scalar(args[:], avg[:], float(BETA_SPEED / P), 0.0, op0=ALU.mult, op1=ALU.add)
            nc.vector.tensor_add(args[:], args[:], worst[:])
            ca_pen = wk.tile([SC, 1], F32)
            nc.vector.tensor_scalar(ca_pen[:], camax[:], float(MAX_CA), 0.0, op0=ALU.subtract, op1=ALU.max)
            nc.vector.tensor_sub(args[:], args[:], ca_pen[:])
            e1 = wk.tile([SC, 1], F32)
            nc.scalar.activation(e1[:], args[:], AF.Exp)
            e2 = wk.tile([SC, 1], F32)
            nc.scalar.activation(e2[:], bmax[:], AF.Exp, scale=-1.0)
            nc.vector.tensor_scalar_max(e2[:], e2[:], 1e-32)
            w = wk.tile([SC, 1], F32)
            nc.vector.tensor_mul(w[:], e1[:], e2[:])

            nc.sync.dma_start(d_diag[:, 0:1], w[:])

            # ---- partial sums ----
            op17 = sml.tile([17, 1], F32, tag="sm")
            nc.tensor.matmul(op17[:], cf[:], w[:], start=True, stop=True)
            o17 = wk.tile([17, 1], F32)
            nc.vector.tensor_copy(o17[:], op17[:])
            nc.sync.dma_start(d_out, o17[:])

    nc.compile()
    return nc


def _host_prep(curve, noise, deltaT, speeds_x, braking_y, bezierM, bezierMd, bezierM2d,
               inner_boundary, inner_normals, outer_boundary, outer_normals):
    f64 = np.float64
    dT = float(deltaT)
    curves = (curve[None].astype(f64) + noise.astype(f64))  # [1024, 8, 2]

    # R [8, 180]
    M = bezierM.astype(f64)
    Md = bezierMd.astype(f64)
    M2d = bezierM2d.astype(f64)
    D1 = _diff_mat(7)
    D1b = _diff_mat(6)[:, :7]
    R = np.zeros((8, 180), f64)
    R[:, 0:60] = M.T
    R[:, 60:120] = (7.0 / dT) * (Md @ D1).T
    R[:, 120:180] = (42.0 / (dT * dT)) * (M2d @ D1b @ D1).T

    # C-shift keeps all scores s1' = |p|^2 - d^2 - Csh strictly negative so
    # FP22 truncation of m (toward zero) can only raise it -> t2 >= 0 exact.
    cmax = max(float(np.abs(curves).max()), 1.0)
    Csh = 2.0 * cmax * cmax + 1.0

    def trunc22(x):
        x32 = np.asarray(x, np.float32).copy()
        u = x32.view(np.uint32)
        u &= np.uint32(0xFFFFF000)
        return x32.astype(f64)

    # boundary tables
    def btab(bpts, bnrm):
        b = bpts.astype(f64)
        n = bnrm.astype(f64)
        b2 = (b * b).sum(1)
        e = (b * n).sum(1)
        A = np.zeros((3, NBP), f64)
        A[0, :NB] = 2 * b[:, 0]
        A[1, :NB] = 2 * b[:, 1]
        A[2, :NB] = -(b2 + Csh)
        A[2, NB:] = -1e30
        G = np.zeros((6, NBP), f64)
        G[0, :NB] = -2 * b[:, 0]
        G[1, :NB] = -2 * b[:, 1]
        b2hi = trunc22(b2 + Csh)
        G[2, :NB] = b2hi
        G[3, :NB] = trunc22(b2 + Csh - b2hi)
        G[2, NB:] = 1e30
        G[4, :] = 1.0
        G[5, :] = 1.0
        T = np.zeros((NBP, 4), f64)
        T[:NB, 0] = e
        T[:NB, 1] = n[:, 0]
        T[:NB, 2] = n[:, 1]
        T[:NB, 3] = 1.0
        return A, G, T, b2.max()

    Ai, Gi, Ti, m2i = btab(inner_boundary, inner_normals)
    Ao, Go, To, m2o = btab(outer_boundary, outer_normals)
    bG = np.concatenate([Gi, Go], 1)

    # select table -> bf16 hi/lo pairs [ehi,elo,nxhi,nxlo,nyhi,nylo,1,0]
    def bf16_rne(x):
        x32 = np.asarray(x, np.float32)
        u = x32.view(np.uint32)
        r = ((u + 0x7FFF + ((u >> 16) & 1)) & 0xFFFF0000).astype(np.uint32)
        return r.view(np.float32).astype(f64)

    tbl = np.concatenate([Ti, To], 0)  # [2048, 4] (e, nx, ny, 1)
    tbl8 = np.zeros((2048, 32), f64)
    for v in range(3):
        hi = bf16_rne(tbl[:, v])
        lo = bf16_rne(tbl[:, v] - hi)
        tbl8[:, 2 * v] = hi
        tbl8[:, 2 * v + 1] = lo
    tbl8[:, 6] = tbl[:, 3]  # the count/ones column
    tb_sb = np.ascontiguousarray(
        tbl8.reshape(2, 8, 128, 32).transpose(2, 0, 1, 3).reshape(128, 512))

    Bmax2 = max(m2i, m2o, 1.0)
    smax = 2.0 * cmax * np.sqrt(Bmax2) + Bmax2 + Csh + 2.0 * cmax * cmax

    # ---- coarse centers (farthest-point sampling) + center-score table ----
    def fps(pts, k):
        d = ((pts - pts[0]) ** 2).sum(1)
        idx = [0]
        for _ in range(k - 1):
            i = int(d.argmax())
            idx.append(i)
            d = np.minimum(d, ((pts - pts[i]) ** 2).sum(1))
        return np.array(idx)

    def kmedoid(pts, k):
        idx = fps(pts, k)
        C = pts[idx]
        for _ in range(5):
            d2 = ((pts[:, None, :] - C[None]) ** 2).sum(-1)
            a = d2.argmin(1)
            for j in range(k):
                msk = a == j
                if msk.any():
                    C[j] = pts[msk].mean(0)
        # snap to nearest actual boundary point (keeps m_hat <= true max)
        d2 = ((pts[:, None, :] - C[None]) ** 2).sum(-1)
        return d2.argmin(0)

    bi = inner_boundary.astype(f64)
    bo = outer_boundary.astype(f64)
    ci_idx = kmedoid(bi, NCC)
    co_idx = kmedoid(bo, NCC)

    # R2c [18, 15*1024]: cols (j4 in 4, bd in 2, c in 128); score = 2c.p - (|c|^2+Csh)
    cents = [bi[ci_idx], bo[co_idx]]  # each [128, 2]
    R2c = np.zeros((18, NQ_C * 1024), f64)
    for q in range(NQ_C):
        for j4 in range(4):
            p = 4 * q + j4
            for bd in range(2):
                cc = cents[bd]  # [128, 2]
                base = q * 1024 + j4 * 256 + bd * NCC
                c2C = (cc ** 2).sum(1) + Csh
                c2hi = np.float16(c2C).astype(f64)
                R2c[0:8, base:base + NCC] = np.outer(M[p, :], 2.0 * cc[:, 0])
                R2c[8:16, base:base + NCC] = np.outer(M[p, :], 2.0 * cc[:, 1])
                R2c[16, base:base + NCC] = -c2hi
                R2c[17, base:base + NCC] = -np.float16(c2C - c2hi).astype(f64)

    # ---- adaptive K from a coverage-gap bound (grid over the query region) ----
    qm = np.sqrt(2.0) * cmax + 0.5
    gs = np.linspace(-qm, qm, 161)
    Q = np.stack(np.meshgrid(gs, gs), -1).reshape(-1, 2)

    def gapbound(b, cidx):
        gap = 0.0
        for lo in range(0, len(Q), 4096):
            d2 = ((Q[lo:lo + 4096, None, :] - b[None]) ** 2).sum(-1)
            gap = max(gap, float((d2[:, cidx].min(1) - d2.min(1)).max()))
        return gap

    gb = 2.0 * max(gapbound(bi, ci_idx), gapbound(bo, co_idx)) + 0.3
    noise = 0.55 * max(smax / 3700.0, 0.05)
    K = float(min(2.0 ** 17 / smax, 60.0 / (gb + noise)))
    theta = float(noise + 3.0 / K)

    # interp constants
    xs = speeds_x.astype(f64)
    ys = braking_y.astype(f64)
    dx = np.diff(xs)
    dx_safe = np.where(dx > 0, dx, 1.0)
    m = np.where(dx > 0, np.diff(ys) / dx_safe, 0.0)

    # per-core shards
    import ml_dtypes
    tb_bf16 = tb_sb.astype(ml_dtypes.bfloat16)
    ins = []
    for c in range(NCORES):
        cs = curves[c * SC:(c + 1) * SC]  # [128, 8, 2]
        cv = np.ascontiguousarray(cs.transpose(2, 1, 0).reshape(16, SC)).astype(np.float32)
        cf17 = np.concatenate([cs.reshape(SC, 16), np.ones((SC, 1))], 1).astype(np.float32)
        cfTc = np.concatenate([cs[:, :, 0].T, cs[:, :, 1].T, np.ones((2, SC))], 0).astype(np.float16)
        ins.append(dict(
            cv=cv, cf17=cf17, cfT=cfTc,
            Rm=R.astype(np.float32), bG=bG.astype(np.float32),
            tb=tb_bf16, R2c=R2c.astype(np.float16),
            Th=np.full((SC, 1), theta, np.float32),
            I8=np.eye(8, dtype=np.float32), I128=np.eye(128, dtype=np.float32),
            Kv=np.full((SC, 1), -K, np.float32),
            ones_row=np.ones((1, SC * P), np.float32),
        ))
    return ins, (xs, dx_safe, m, float(ys[0]), K)


def kernel(curve, noise, deltaT, speeds_x, braking_y, bezierM, bezierMd, bezierM2d,
           inner_boundary, inner_normals, outer_boundary, outer_normals):
    in_maps, (xs, dxs, ms, y0, K) = _host_prep(
        curve, noise, deltaT, speeds_x, braking_y, bezierM, bezierMd, bezierM2d,
        inner_boundary, inner_normals, outer_boundary, outer_normals)

    key = (tuple(np.round(xs, 9)), tuple(np.round(ms, 9)), round(y0, 9))
    if key not in _cache:
        _cache.clear()
        _cache[key] = _build_program(xs, dxs, ms, y0)
    nc = _cache[key]

    res = bass_utils.run_bass_kernel_spmd(nc, in_maps, core_ids=list(range(NCORES)))
    outs = res.results
    num = np.zeros(16, np.float64)
    Z = 0.0
    for c in range(NCORES):
        o = np.asarray(outs[c]["out17"]).reshape(17)
        num += o[:16].astype(np.float64)
        Z += float(o[16])
    return (num / Z).reshape(8, 2).astype(np.float32)


if __name__ == "__main__":
    import reference
    inp = {k: np.asarray(v) for k, v in reference.setup_inputs().items()}
    out = kernel(**inp)
    exp = np.asarray(reference.reference(**reference.setup_inputs()))
    err = np.abs(out - exp).max() / (np.abs(exp).max() + 1e-12)
    print("Relative error:", err)



# revision 28
# speedup vs baseline: 1.0890x; 1.0890x over previous
"""Bayesian curve filter kernel for Trainium2 (8 NeuronCores, SPMD).

Sharding: data-parallel over the 1024 Monte-Carlo samples -> 128 per core
(exactly the SBUF partition count; samples live on partitions).

Device algorithm per core (all fp32):
  1. out1 = curves^T @ R : per-sample curve points / velocity / accel
     [128s, 180] (cols 0-59 pts, 60-119 v_t, 120-179 a_t) per dim d.
  2. speeds / centripetal / braking-interp pipeline on [128, 60] tiles.
  3. Boundary nearest-neighbor (the heavy part), soft-select formulation:
       s1[s,b]   = 2 x.b - |b|^2            (argmax_b s1 == argmin_b d2)
       m[s]      = max_b s1                 (DVE reduce over 2x1000 scores)
       t[b,s]    = m - s1 >= 0              (PE re-emission, [b,s] layout)
       H         = exp(-K t)                (ACT; ~one-hot at the argmin)
       sel[4,s]  = sum_b H_b * (e_b, cnx_b, cny_b, 1)   (PE contraction)
       dist      = (sel0 - px*sel1 - py*sel2) / sel3
  4. Per-sample log-score -> w; partial (sum_s w*curve_s, sum_s w) via a
     final [128,17]x[128,1] matmul -> [17] per core; host sums across the
     8 cores and divides (softmax normalization cancels globally).
"""

import os
import numpy as np
from math import comb

import concourse.bass as bass
import concourse.bacc as bacc
import concourse.mybir as mybir
from concourse import tile
from concourse import bass_utils

F32 = mybir.dt.float32
F32R = mybir.dt.float32r
BF16 = mybir.dt.bfloat16
F16 = mybir.dt.float16
U32 = mybir.dt.uint32
ALU = mybir.AluOpType
AF = mybir.ActivationFunctionType
AX = mybir.AxisListType


def _r(ap):
    return ap.bitcast(F32R)

NCORES = 8
S_FULL = 1024
SC = 128          # samples per core
P = 60            # points per curve
NB = 1000         # boundary points per boundary
NBP = 1024        # padded
ORD = 7           # bezier order
BETA_SPEED = 0.1
MAX_CA = 19.6
NSEG = 19         # interp segments (20 knots)
NCC = 128         # coarse centers per boundary (level-1 max estimate)
NQ_C = 15         # quads

_cache = {}


def _diff_mat(n):
    # D [n, n+1]: (D @ c)[k] = c[k+1] - c[k]
    D = np.zeros((n, n + 1), np.float64)
    for k in range(n):
        D[k, k] = -1.0
        D[k, k + 1] = 1.0
    return D


def _build_program(interp_x, interp_dx, interp_m, y0):
    """Builds the bass program. interp constants are baked as immediates."""
    nc = bacc.Bacc("TRN2", target_bir_lowering=False, debug=False, enable_asserts=False)

    # ---- DRAM I/O ----
    d_cv = nc.dram_tensor("cv", [16, SC], F32, kind="ExternalInput").ap()       # curvesT: rows 0-7 x-coefs, 8-15 y
    d_cf = nc.dram_tensor("cf17", [SC, 17], F32, kind="ExternalInput").ap()     # curves flat + ones col
    d_R = nc.dram_tensor("Rm", [8, 180], F32, kind="ExternalInput").ap()
    d_bG = nc.dram_tensor("bG", [6, 2 * NBP], F32R, kind="ExternalInput").ap()   # em2 lhsT [-2bx;-2by;b2Chi;b2Clo;1;1]
    d_tb = nc.dram_tensor("tb", [SC, 512], BF16, kind="ExternalInput").ap()       # select lhsT chunks [ehi,elo,nxhi,nxlo,nyhi,nylo,1,0]
    d_Th = nc.dram_tensor("Th", [SC, 1], F32, kind="ExternalInput").ap()  # is_le threshold
    d_I8 = nc.dram_tensor("I8", [8, 8], F32, kind="ExternalInput").ap()
    d_I128 = nc.dram_tensor("I128", [SC, SC], F32, kind="ExternalInput").ap()
    d_Kv = nc.dram_tensor("Kv", [SC, 1], F32, kind="ExternalInput").ap()  # -K replicated
    d_ones = nc.dram_tensor("ones_row", [1, P * SC], F32R, kind="ExternalInput").ap()
    d_cfT = nc.dram_tensor("cfT", [18, SC], F16, kind="ExternalInput").ap()     # [X8; Y8; 1; 1] per sample
    d_R2c = nc.dram_tensor("R2c", [18, NQ_C * 1024], F16, kind="ExternalInput").ap()  # center-score table
    d_out = nc.dram_tensor("out17", [17, 1], F32, kind="ExternalOutput").ap()
    d_diag = nc.dram_tensor("diag", [SC, 8], F32, kind="ExternalOutput").ap()

    NQ = 15  # quads of p (4 p's each -> 512-wide sp blocks)

    with tile.TileContext(nc) as tc:
        with (
            tc.tile_pool(name="cst", bufs=1) as cst,
            tc.tile_pool(name="paug", bufs=1) as paugp,
            tc.tile_pool(name="selc", bufs=1) as selcp,
            tc.tile_pool(name="selc2", bufs=3) as selcp2,
            tc.tile_pool(name="hbuf", bufs=4) as hbuf,
            tc.tile_pool(name="wk", bufs=4) as wk,
            tc.tile_pool(name="m2", bufs=4) as m2p,
            tc.tile_pool(name="big", bufs=2, space="PSUM") as big,
            tc.tile_pool(name="sml", bufs=3, space="PSUM") as sml,
        ):
            # ---- load constants (replicated at partition bases 0/32/64/96) ----
            cvx = cst.tile([8, SC], F32)
            nc.sync.dma_start(cvx[:], d_cv[0:8, :])
            cvy = cst.tile([8, SC], F32)
            nc.sync.dma_start(cvy[:], d_cv[8:16, :])
            cf = cst.tile([SC, 17], F32)
            nc.sync.dma_start(cf[:], d_cf)
            Rm = cst.tile([8, 180], F32)
            nc.sync.dma_start(Rm[:], d_R)
            cfT = cst.tile([18, SC], F16)
            nc.sync.dma_start(cfT[:], d_cfT)
            R2c = cst.tile([18, NQ_C * 1024], F16)
            nc.sync.dma_start(R2c[:], d_R2c)
            I128 = cst.tile([SC, SC], F32)
            nc.sync.dma_start(I128[:], d_I128)
            bG = cst.tile([102, 2 * NBP], F32R)
            I8r = cst.tile([8, 8], F32)
            nc.scalar.dma_start(I8r[:], d_I8)
            for j in range(4):
                nc.scalar.dma_start(bG[32 * j:32 * j + 6, :], d_bG)
            tb = cst.tile([SC, 512], BF16)
            thv = cst.tile([SC, 1], F32)
            nc.scalar.dma_start(thv[:], d_Th)
            b25 = cst.tile([SC, 1], F32)
            nc.vector.memset(b25[:], -25.0)
            nc.scalar.dma_start(tb[:], d_tb)
            Kv = cst.tile([SC, 1], F32)
            nc.scalar.dma_start(Kv[:], d_Kv)

            # ---- pts/vel/accel in [s, col] layout ----
            o1x = sml.tile([SC, 180], F32, tag="sm")
            nc.tensor.matmul(o1x[:], cvx[:], Rm[:], start=True, stop=True)
            o1y = sml.tile([SC, 180], F32, tag="sm")
            nc.tensor.matmul(o1y[:], cvy[:], Rm[:], start=True, stop=True)
            ox = cst.tile([SC, 180], F32)
            nc.vector.tensor_copy(ox[:], o1x[:])
            oy = cst.tile([SC, 180], F32)
            nc.vector.tensor_copy(oy[:], o1y[:])

            # ---- pts in [p, s] layout -> paug rows ----
            ptx = sml.tile([P, SC], F32, tag="sm")
            nc.tensor.matmul(ptx[:], Rm[:, 0:P], cvx[:], start=True, stop=True)
            pty = sml.tile([P, SC], F32, tag="sm")
            nc.tensor.matmul(pty[:], Rm[:, 0:P], cvy[:], start=True, stop=True)
            ptxs = cst.tile([P, SC], F32)
            nc.vector.tensor_copy(ptxs[:], ptx[:])
            ptys = cst.tile([P, SC], F32)
            nc.vector.tensor_copy(ptys[:], pty[:])

            pgi = paugp.tile([102, P * SC], F32R)  # rows 32j..+5: [px; py; 1; 1; mhi_in; mlo_in]
            pgo = paugp.tile([102, P * SC], F32R)  # rows 32j..+5: [px; py; 1; 1; mhi_out; mlo_out]
            qs = [nc.sync, nc.scalar, nc.gpsimd]
            di = 0
            for j in range(4):
                for pg in (pgi, pgo):
                    qs[di % 3].dma_start(pg[32 * j:32 * j + 1, :].rearrange("o (p s) -> o p s", p=P), _r(ptxs[:])); di += 1
                    qs[di % 3].dma_start(pg[32 * j + 1:32 * j + 2, :].rearrange("o (p s) -> o p s", p=P), _r(ptys[:])); di += 1
                    qs[di % 3].dma_start(pg[32 * j + 2:32 * j + 3, :], d_ones); di += 1
                    qs[di % 3].dma_start(pg[32 * j + 3:32 * j + 4, :], d_ones); di += 1

            # ---- boundary: per-quad pipeline, 4-way row/col tiled matmuls ----
            dTs = selcp.tile([SC, NQ * 64], F32)
            m2qs = [None] * NQ

            def em1(q):
                m2q = m2p.tile([SC, 64], F32, tag="m2")  # 4x copies: mhi blk 0:32, mlo blk 32:64
                m2qs[q] = m2q
                # coarse center scores: [s, (p4, bd2, c128)] via K=18 matmul
                for hh in range(2):
                    cs = sml.tile([SC, 512], F32, tag="sm")
                    nc.tensor.matmul(cs[:], cfT[:],
                                     R2c[:, q * 1024 + hh * 512: q * 1024 + (hh + 1) * 512],
                                     start=True, stop=True)
                    nc.vector.tensor_reduce(
                        m2q[:, 32 + 4 * hh: 36 + 4 * hh],
                        cs[:].rearrange("s (g c) -> s g c", c=NCC),
                        axis=AX.X, op=ALU.max)
                # mhi = trunc22(m) -> cols 0:8; mlo = trunc22(m - mhi) -> cols 32:40
                nc.vector.tensor_scalar(
                    m2q[:, 0:8].bitcast(U32), m2q[:, 32:40].bitcast(U32),
                    0xFFFFF000, None, op0=ALU.bitwise_and)
                nc.vector.tensor_sub(m2q[:, 32:40], m2q[:, 32:40], m2q[:, 0:8])
                nc.vector.tensor_scalar(
                    m2q[:, 32:40].bitcast(U32), m2q[:, 32:40].bitcast(U32),
                    0xFFFFF000, None, op0=ALU.bitwise_and)
                # replicate each 8-vec 3 more times within its 32-block
                nc.vector.tensor_copy(
                    m2q[:, 8:32].rearrange("s (r v) -> s r v", v=8),
                    m2q[:, 0:8].rearrange("s (r v) -> s r v", r=1).to_broadcast((SC, 3, 8)))
                nc.vector.tensor_copy(
                    m2q[:, 40:64].rearrange("s (r v) -> s r v", v=8),
                    m2q[:, 32:40].rearrange("s (r v) -> s r v", r=1).to_broadcast((SC, 3, 8)))
                mT = sml.tile([64, SC], F32, tag="sm")
                nc.tensor.matmul(mT[:], m2q[:], I128[:], start=True, stop=True)
                mTs = wk.tile([64, SC], F32, tag="mts")
                nc.scalar.copy(mTs[:], mT[:])
                qc = slice(q * 512, (q + 1) * 512)
                nc.sync.dma_start(
                    pgi[4:102:32, qc].rearrange("g (j2 s) -> g j2 s", j2=4), _r(mTs[0:32:2, :]))
                nc.scalar.dma_start(
                    pgo[4:102:32, qc].rearrange("g (j2 s) -> g j2 s", j2=4), _r(mTs[1:32:2, :]))
                nc.sync.dma_start(
                    pgi[5:102:32, qc].rearrange("g (j2 s) -> g j2 s", j2=4), _r(mTs[32:64:2, :]))
                nc.scalar.dma_start(
                    pgo[5:102:32, qc].rearrange("g (j2 s) -> g j2 s", j2=4), _r(mTs[33:64:2, :]))

            def em2(q):
                selc = selcp2.tile([8, 1024], F32, tag="selc")
                for bd in range(2):
                    pg = pgi if bd == 0 else pgo
                    sp = sml.tile([40, 512], F32, tag="sm")
                    for hw in range(4):  # windows of 2 chunks
                        t2 = big.tile([SC, NBP], F32, tag="big")
                        for cc in range(2):
                            c = 2 * hw + cc
                            g = 32 * (c % 4)
                            nc.tensor.matmul(
                                t2[:, cc * 512:(cc + 1) * 512],
                                bG[g:g + 6, bd * NBP + c * SC: bd * NBP + (c + 1) * SC],
                                pg[g:g + 6, q * 512:(q + 1) * 512],
                                start=True, stop=True, tile_position=(g, 0))
                        Ht = hbuf.tile([SC, NBP], BF16, tag="h")
                        if bd == 0:
                            nc.scalar.activation(Ht[:], t2[:], AF.Exp, scale=Kv[:], bias=b25[:])
                        else:
                            nc.vector.tensor_scalar(Ht[:], t2[:], thv[:], None, op0=ALU.is_le)
                        for cc in range(2):
                            c = 2 * hw + cc
                            cg = 32 * (c % 2)
                            nc.tensor.matmul(
                                sp[cg:cg + 8, :], tb[:, (bd * 8 + c) * 32:(bd * 8 + c) * 32 + 8],
                                Ht[:, cc * 512:(cc + 1) * 512],
                                start=(c < 2), stop=(c >= 6), tile_position=(0, cg))
                    nc.scalar.copy(selc[:, bd * 512:(bd + 1) * 512], sp[0:8, :])
                    nc.vector.tensor_add(selc[:, bd * 512:(bd + 1) * 512],
                                         selc[:, bd * 512:(bd + 1) * 512], sp[32:40, :])
                dTq = sml.tile([SC, 64], F32, tag="sm")
                for j4 in range(4):
                    for bd in range(2):
                        off = bd * 512 + j4 * SC
                        nc.tensor.matmul(
                            dTq[:, j4 * 16 + bd * 8: j4 * 16 + (bd + 1) * 8],
                            selc[:, off: off + SC], I8r[:],
                            start=True, stop=True)
                nc.scalar.copy(dTs[:, q * 64:(q + 1) * 64], dTq[:])

            for q in range(NQ + 3):
                if q < NQ:
                    em1(q)
                if q >= 3:
                    em2(q - 3)

            # ---- speeds / accel pipeline [128, 60] ----
            vx, vy, ax_, ay = (ox[:, 60:120], oy[:, 60:120], ox[:, 120:180], oy[:, 120:180])
            spd2 = wk.tile([SC, P], F32)
            nc.vector.tensor_mul(spd2[:], vx, vx)
            t0 = wk.tile([SC, P], F32)
            nc.vector.tensor_mul(t0[:], vy, vy)
            nc.vector.tensor_add(spd2[:], spd2[:], t0[:])
            spd = wk.tile([SC, P], F32)
            nc.scalar.activation(spd[:], spd2[:], AF.Sqrt)
            rspd = wk.tile([SC, P], F32)
            nc.vector.reciprocal(rspd[:], spd[:])
            adv = wk.tile([SC, P], F32)
            nc.vector.tensor_mul(adv[:], ax_, vx)
            nc.vector.tensor_mul(t0[:], ay, vy)
            nc.vector.tensor_add(adv[:], adv[:], t0[:])
            lin = wk.tile([SC, P], F32)
            nc.vector.tensor_mul(lin[:], adv[:], rspd[:])
            a2 = wk.tile([SC, P], F32)
            nc.vector.tensor_mul(a2[:], ax_, ax_)
            nc.vector.tensor_mul(t0[:], ay, ay)
            nc.vector.tensor_add(a2[:], a2[:], t0[:])
            nc.vector.tensor_mul(t0[:], lin[:], lin[:])
            nc.vector.tensor_sub(a2[:], a2[:], t0[:])  # ca^2 (may be ~-eps)
            camax2 = wk.tile([SC, 1], F32)
            nc.vector.tensor_reduce(camax2[:], a2[:], axis=AX.X, op=ALU.max)
            nc.vector.tensor_scalar_max(camax2[:], camax2[:], 0.0)
            camax = wk.tile([SC, 1], F32)
            nc.scalar.activation(camax[:], camax2[:], AF.Sqrt)

            avg = wk.tile([SC, 1], F32)
            nc.vector.tensor_reduce(avg[:], spd[:], axis=AX.X, op=ALU.add)

            # braking interp: bl = y0 + sum_i m_i * clip(spd - x_i, 0, dx_i)
            bl = wk.tile([SC, P], F32)
            nc.vector.memset(bl[:], float(y0))
            ti = wk.tile([SC, P], F32)
            for i in range(NSEG):
                nc.vector.tensor_scalar(ti[:], spd[:], float(interp_x[i]), 0.0, op0=ALU.subtract, op1=ALU.max)
                nc.vector.tensor_scalar(ti[:], ti[:], float(interp_dx[i]), float(interp_m[i]), op0=ALU.min, op1=ALU.mult)
                nc.vector.tensor_add(bl[:], bl[:], ti[:])
            bv = wk.tile([SC, P], F32)
            nc.vector.tensor_sub(bv[:], lin[:], bl[:])
            worst = wk.tile([SC, 1], F32)
            nc.vector.tensor_reduce(worst[:], bv[:], axis=AX.X, op=ALU.min)
            nc.vector.tensor_scalar_min(worst[:], worst[:], 0.0)

            # ---- phase C: dist + maxes (transposes done per-quad above) ----
            bmax = wk.tile([SC, 1], F32)
            for bd in range(2):
                Se = wk.tile([SC, P], F32, tag="c0")
                nc.vector.tensor_add(Se[:], dTs[:, bd * 8 + 0:960:16], dTs[:, bd * 8 + 1:960:16])
                Scx = wk.tile([SC, P], F32, tag="c1")
                nc.vector.tensor_add(Scx[:], dTs[:, bd * 8 + 2:960:16], dTs[:, bd * 8 + 3:960:16])
                Scy = wk.tile([SC, P], F32, tag="c2")
                nc.vector.tensor_add(Scy[:], dTs[:, bd * 8 + 4:960:16], dTs[:, bd * 8 + 5:960:16])
                Sn = dTs[:, bd * 8 + 6:960:16]
                Se = Se[:]
                Scx = Scx[:]
                Scy = Scy[:]
                n1 = wk.tile([SC, P], F32, tag="d1")
                nc.vector.tensor_mul(n1[:], ox[:, 0:P], Scx)
                n2 = wk.tile([SC, P], F32, tag="d2")
                nc.vector.tensor_mul(n2[:], oy[:, 0:P], Scy)
                nc.vector.tensor_sub(n1[:], Se, n1[:])
                nc.vector.tensor_sub(n1[:], n1[:], n2[:])
                rs = wk.tile([SC, P], F32, tag="d3")
                nc.vector.reciprocal(rs[:], Sn)
                nc.vector.tensor_mul(n1[:], n1[:], rs[:])
                dm = wk.tile([SC, 1], F32, tag="d4")
                nc.vector.tensor_reduce(dm[:], n1[:], axis=AX.X, op=ALU.max)
                if bd == 0:
                    nc.vector.tensor_copy(bmax[:], dm[:])
                else:
                    nc.vector.tensor_max(bmax[:], bmax[:], dm[:])
            nc.vector.tensor_scalar_max(bmax[:], bmax[:], 0.0)

            # ---- per-sample scores -> w ----
            args = wk.tile([SC, 1], F32)
            nc.vector.tensor_scalar(args[:], avg[:], float(BETA_SPEED / P), 0.0, op0=ALU.mult, op1=ALU.add)
            nc.vector.tensor_add(args[:], args[:], worst[:])
            ca_pen = wk.tile([SC, 1], F32)
            nc.vector.tensor_scalar(ca_pen[:], camax[:], float(MAX_CA), 0.0, op0=ALU.subtract, op1=ALU.max)
            nc.vector.tensor_sub(args[:], args[:], ca_pen[:])
            e1 = wk.tile([SC, 1], F32)
            nc.scalar.activation(e1[:], args[:], AF.Exp)
            e2 = wk.tile([SC, 1], F32)
            nc.scalar.activation(e2[:], bmax[:], AF.Exp, scale=-1.0)
            nc.vector.tensor_scalar_max(e2[:], e2[:], 1e-32)
            w = wk.tile([SC, 1], F32)
            nc.vector.tensor_mul(w[:], e1[:], e2[:])

            nc.sync.dma_start(d_diag[:, 0:1], w[:])

            # ---- partial sums ----
            op17 = sml.tile([17, 1], F32, tag="sm")
            nc.tensor.matmul(op17[:], cf[:], w[:], start=True, stop=True)
            o17 = wk.tile([17, 1], F32)
            nc.vector.tensor_copy(o17[:], op17[:])
            nc.sync.dma_start(d_out, o17[:])

    nc.compile()
    return nc


def _host_prep(curve, noise, deltaT, speeds_x, braking_y, bezierM, bezierMd, bezierM2d,
               inner_boundary, inner_normals, outer_boundary, outer_normals):
    f64 = np.float64
    dT = float(deltaT)
    curves = (curve[None].astype(f64) + noise.astype(f64))  # [1024, 8, 2]

    # R [8, 180]
    M = bezierM.astype(f64)
    Md = bezierMd.astype(f64)
    M2d = bezierM2d.astype(f64)
    D1 = _diff_mat(7)
    D1b = _diff_mat(6)[:, :7]
    R = np.zeros((8, 180), f64)
    R[:, 0:60] = M.T
    R[:, 60:120] = (7.0 / dT) * (Md @ D1).T
    R[:, 120:180] = (42.0 / (dT * dT)) * (M2d @ D1b @ D1).T

    # C-shift keeps all scores s1' = |p|^2 - d^2 - Csh strictly negative so
    # FP22 truncation of m (toward zero) can only raise it -> t2 >= 0 exact.
    cmax = max(float(np.abs(curves).max()), 1.0)
    Csh = 2.0 * cmax * cmax + 1.0

    def trunc22(x):
        x32 = np.asarray(x, np.float32).copy()
        u = x32.view(np.uint32)
        u &= np.uint32(0xFFFFF000)
        return x32.astype(f64)

    # boundary tables
    def btab(bpts, bnrm):
        b = bpts.astype(f64)
        n = bnrm.astype(f64)
        b2 = (b * b).sum(1)
        e = (b * n).sum(1)
        A = np.zeros((3, NBP), f64)
        A[0, :NB] = 2 * b[:, 0]
        A[1, :NB] = 2 * b[:, 1]
        A[2, :NB] = -(b2 + Csh)
        A[2, NB:] = -1e30
        G = np.zeros((6, NBP), f64)
        G[0, :NB] = -2 * b[:, 0]
        G[1, :NB] = -2 * b[:, 1]
        b2hi = trunc22(b2 + Csh)
        G[2, :NB] = b2hi
        G[3, :NB] = trunc22(b2 + Csh - b2hi)
        G[2, NB:] = 1e30
        G[4, :] = 1.0
        G[5, :] = 1.0
        T = np.zeros((NBP, 4), f64)
        T[:NB, 0] = e
        T[:NB, 1] = n[:, 0]
        T[:NB, 2] = n[:, 1]
        T[:NB, 3] = 1.0
        return A, G, T, b2.max()

    Ai, Gi, Ti, m2i = btab(inner_boundary, inner_normals)
    Ao, Go, To, m2o = btab(outer_boundary, outer_normals)
    bG = np.concatenate([Gi, Go], 1)

    # select table -> bf16 hi/lo pairs [ehi,elo,nxhi,nxlo,nyhi,nylo,1,0]
    def bf16_rne(x):
        x32 = np.asarray(x, np.float32)
        u = x32.view(np.uint32)
        r = ((u + 0x7FFF + ((u >> 16) & 1)) & 0xFFFF0000).astype(np.uint32)
        return r.view(np.float32).astype(f64)

    tbl = np.concatenate([Ti, To], 0)  # [2048, 4] (e, nx, ny, 1)
    tbl8 = np.zeros((2048, 32), f64)
    for v in range(3):
        hi = bf16_rne(tbl[:, v])
        lo = bf16_rne(tbl[:, v] - hi)
        tbl8[:, 2 * v] = hi
        tbl8[:, 2 * v + 1] = lo
    tbl8[:, 6] = tbl[:, 3]  # the count/ones column
    tb_sb = np.ascontiguousarray(
        tbl8.reshape(2, 8, 128, 32).transpose(2, 0, 1, 3).reshape(128, 512))

    Bmax2 = max(m2i, m2o, 1.0)
    smax = 2.0 * cmax * np.sqrt(Bmax2) + Bmax2 + Csh + 2.0 * cmax * cmax

    # ---- coarse centers (farthest-point sampling) + center-score table ----
    def fps(pts, k):
        d = ((pts - pts[0]) ** 2).sum(1)
        idx = [0]
        for _ in range(k - 1):
            i = int(d.argmax())
            idx.append(i)
            d = np.minimum(d, ((pts - pts[i]) ** 2).sum(1))
        return np.array(idx)

    def kmedoid(pts, k):
        idx = fps(pts, k)
        C = pts[idx]
        for _ in range(5):
            d2 = ((pts[:, None, :] - C[None]) ** 2).sum(-1)
            a = d2.argmin(1)
            for j in range(k):
                msk = a == j
                if msk.any():
                    C[j] = pts[msk].mean(0)
        # snap to nearest actual boundary point (keeps m_hat <= true max)
        d2 = ((pts[:, None, :] - C[None]) ** 2).sum(-1)
        return d2.argmin(0)

    bi = inner_boundary.astype(f64)
    bo = outer_boundary.astype(f64)
    ci_idx = kmedoid(bi, NCC)
    co_idx = kmedoid(bo, NCC)

    # R2c [18, 15*1024]: cols (j4 in 4, bd in 2, c in 128); score = 2c.p - (|c|^2+Csh)
    cents = [bi[ci_idx], bo[co_idx]]  # each [128, 2]
    R2c = np.zeros((18, NQ_C * 1024), f64)
    for q in range(NQ_C):
        for j4 in range(4):
            p = 4 * q + j4
            for bd in range(2):
                cc = cents[bd]  # [128, 2]
                base = q * 1024 + j4 * 256 + bd * NCC
                c2C = (cc ** 2).sum(1) + Csh
                c2hi = np.float16(c2C).astype(f64)
                R2c[0:8, base:base + NCC] = np.outer(M[p, :], 2.0 * cc[:, 0])
                R2c[8:16, base:base + NCC] = np.outer(M[p, :], 2.0 * cc[:, 1])
                R2c[16, base:base + NCC] = -c2hi
                R2c[17, base:base + NCC] = -np.float16(c2C - c2hi).astype(f64)

    # ---- adaptive K from a coverage-gap bound (grid over the query region) ----
    qm = np.sqrt(2.0) * cmax + 0.5
    gs = np.linspace(-qm, qm, 161)
    Q = np.stack(np.meshgrid(gs, gs), -1).reshape(-1, 2)

    def gapbound(b, cidx):
        gap = 0.0
        for lo in range(0, len(Q), 4096):
            d2 = ((Q[lo:lo + 4096, None, :] - b[None]) ** 2).sum(-1)
            gap = max(gap, float((d2[:, cidx].min(1) - d2.min(1)).max()))
        return gap

    gb = 2.0 * max(gapbound(bi, ci_idx), gapbound(bo, co_idx)) + 0.3
    noise = 0.55 * max(smax / 3700.0, 0.05)
    K = float(min(2.0 ** 17 / smax, 60.0 / (gb + noise)))
    theta = float(noise + 3.0 / K)

    # interp constants
    xs = speeds_x.astype(f64)
    ys = braking_y.astype(f64)
    dx = np.diff(xs)
    dx_safe = np.where(dx > 0, dx, 1.0)
    m = np.where(dx > 0, np.diff(ys) / dx_safe, 0.0)

    # per-core shards
    import ml_dtypes
    tb_bf16 = tb_sb.astype(ml_dtypes.bfloat16)
    ins = []
    for c in range(NCORES):
        cs = curves[c * SC:(c + 1) * SC]  # [128, 8, 2]
        cv = np.ascontiguousarray(cs.transpose(2, 1, 0).reshape(16, SC)).astype(np.float32)
        cf17 = np.concatenate([cs.reshape(SC, 16), np.ones((SC, 1))], 1).astype(np.float32)
        cfTc = np.concatenate([cs[:, :, 0].T, cs[:, :, 1].T, np.ones((2, SC))], 0).astype(np.float16)
        ins.append(dict(
            cv=cv, cf17=cf17, cfT=cfTc,
            Rm=R.astype(np.float32), bG=bG.astype(np.float32),
            tb=tb_bf16, R2c=R2c.astype(np.float16),
            Th=np.full((SC, 1), theta, np.float32),
            I8=np.eye(8, dtype=np.float32), I128=np.eye(128, dtype=np.float32),
            Kv=np.full((SC, 1), -K, np.float32),
            ones_row=np.ones((1, SC * P), np.float32),
        ))
    return ins, (xs, dx_safe, m, float(ys[0]), K)


def kernel(curve, noise, deltaT, speeds_x, braking_y, bezierM, bezierMd, bezierM2d,
           inner_boundary, inner_normals, outer_boundary, outer_normals):
    in_maps, (xs, dxs, ms, y0, K) = _host_prep(
        curve, noise, deltaT, speeds_x, braking_y, bezierM, bezierMd, bezierM2d,
        inner_boundary, inner_normals, outer_boundary, outer_normals)

    key = (tuple(np.round(xs, 9)), tuple(np.round(ms, 9)), round(y0, 9))
    if key not in _cache:
        _cache.clear()
        _cache[key] = _build_program(xs, dxs, ms, y0)
    nc = _cache[key]

    res = bass_utils.run_bass_kernel_spmd(nc, in_maps, core_ids=list(range(NCORES)))
    outs = res.results
    num = np.zeros(16, np.float64)
    Z = 0.0
    for c in range(NCORES):
        o = np.asarray(outs[c]["out17"]).reshape(17)
        num += o[:16].astype(np.float64)
        Z += float(o[16])
    return (num / Z).reshape(8, 2).astype(np.float32)


if __name__ == "__main__":
    import reference
    inp = {k: np.asarray(v) for k, v in reference.setup_inputs().items()}
    out = kernel(**inp)
    exp = np.asarray(reference.reference(**reference.setup_inputs()))
    err = np.abs(out - exp).max() / (np.abs(exp).max() + 1e-12)
    print("Relative error:", err)



# revision 30
# speedup vs baseline: 1.2199x; 1.1202x over previous
"""Bayesian curve filter kernel for Trainium2 (8 NeuronCores, SPMD).

Sharding: data-parallel over the 1024 Monte-Carlo samples -> 128 per core
(exactly the SBUF partition count; samples live on partitions).

Device algorithm per core (all fp32):
  1. out1 = curves^T @ R : per-sample curve points / velocity / accel
     [128s, 180] (cols 0-59 pts, 60-119 v_t, 120-179 a_t) per dim d.
  2. speeds / centripetal / braking-interp pipeline on [128, 60] tiles.
  3. Boundary nearest-neighbor (the heavy part), soft-select formulation:
       s1[s,b]   = 2 x.b - |b|^2            (argmax_b s1 == argmin_b d2)
       m[s]      = max_b s1                 (DVE reduce over 2x1000 scores)
       t[b,s]    = m - s1 >= 0              (PE re-emission, [b,s] layout)
       H         = exp(-K t)                (ACT; ~one-hot at the argmin)
       sel[4,s]  = sum_b H_b * (e_b, cnx_b, cny_b, 1)   (PE contraction)
       dist      = (sel0 - px*sel1 - py*sel2) / sel3
  4. Per-sample log-score -> w; partial (sum_s w*curve_s, sum_s w) via a
     final [128,17]x[128,1] matmul -> [17] per core; host sums across the
     8 cores and divides (softmax normalization cancels globally).
"""

import os
import numpy as np
from math import comb

import concourse.bass as bass
import concourse.bacc as bacc
import concourse.mybir as mybir
from concourse import tile
from concourse import bass_utils

F32 = mybir.dt.float32
F32R = mybir.dt.float32r
BF16 = mybir.dt.bfloat16
F16 = mybir.dt.float16
U32 = mybir.dt.uint32
ALU = mybir.AluOpType
AF = mybir.ActivationFunctionType
AX = mybir.AxisListType


def _r(ap):
    return ap.bitcast(F32R)

NCORES = 8
S_FULL = 1024
SC = 128          # samples per core
P = 60            # points per curve
NB = 1000         # boundary points per boundary
NBP = 1024        # padded
ORD = 7           # bezier order
BETA_SPEED = 0.1
MAX_CA = 19.6
NSEG = 19         # interp segments (20 knots)
NCC = 128         # coarse centers per boundary (level-1 max estimate)
NQ_C = 15         # quads

_cache = {}


def _diff_mat(n):
    # D [n, n+1]: (D @ c)[k] = c[k+1] - c[k]
    D = np.zeros((n, n + 1), np.float64)
    for k in range(n):
        D[k, k] = -1.0
        D[k, k + 1] = 1.0
    return D


def _build_program(interp_x, interp_dx, interp_m, y0):
    """Builds the bass program. interp constants are baked as immediates."""
    nc = bacc.Bacc("TRN2", target_bir_lowering=False, debug=False, enable_asserts=False)

    # ---- DRAM I/O ----
    d_cv = nc.dram_tensor("cv", [16, SC], F32, kind="ExternalInput").ap()       # curvesT: rows 0-7 x-coefs, 8-15 y
    d_cf = nc.dram_tensor("cf17", [SC, 17], F32, kind="ExternalInput").ap()     # curves flat + ones col
    d_R = nc.dram_tensor("Rm", [8, 180], F32, kind="ExternalInput").ap()
    d_bG = nc.dram_tensor("bG", [6, 2 * NBP], F16, kind="ExternalInput").ap()   # em2 lhsT [-2bx;-2by;b2Chi;b2Clo;1;1]
    d_tb = nc.dram_tensor("tb", [SC, 512], BF16, kind="ExternalInput").ap()       # select lhsT chunks [ehi,elo,nxhi,nxlo,nyhi,nylo,1,0]
    d_Th = nc.dram_tensor("Th", [SC, 1], F32, kind="ExternalInput").ap()  # is_le threshold
    d_I8 = nc.dram_tensor("I8", [8, 8], F32, kind="ExternalInput").ap()
    d_I128 = nc.dram_tensor("I128", [SC, SC], F16, kind="ExternalInput").ap()
    d_Kv = nc.dram_tensor("Kv", [SC, 1], F32, kind="ExternalInput").ap()  # -K replicated
    d_ones = nc.dram_tensor("ones_row", [1, P * SC], F16, kind="ExternalInput").ap()
    d_cfT = nc.dram_tensor("cfT", [18, SC], F16, kind="ExternalInput").ap()     # [X8; Y8; 1; 1] per sample
    d_R2c = nc.dram_tensor("R2c", [18, NQ_C * 1024], F16, kind="ExternalInput").ap()  # center-score table
    d_out = nc.dram_tensor("out17", [17, 1], F32, kind="ExternalOutput").ap()
    d_diag = nc.dram_tensor("diag", [SC, 8], F32, kind="ExternalOutput").ap()

    NQ = 15  # quads of p (4 p's each -> 512-wide sp blocks)

    with tile.TileContext(nc) as tc:
        with (
            tc.tile_pool(name="cst", bufs=1) as cst,
            tc.tile_pool(name="paug", bufs=1) as paugp,
            tc.tile_pool(name="selc", bufs=1) as selcp,
            tc.tile_pool(name="selc2", bufs=3) as selcp2,
            tc.tile_pool(name="hbuf", bufs=4) as hbuf,
            tc.tile_pool(name="wk", bufs=4) as wk,
            tc.tile_pool(name="m2", bufs=4) as m2p,
            tc.tile_pool(name="big", bufs=2, space="PSUM") as big,
            tc.tile_pool(name="sml", bufs=3, space="PSUM") as sml,
        ):
            # ---- load constants (replicated at partition bases 0/32/64/96) ----
            cvx = cst.tile([8, SC], F32)
            nc.sync.dma_start(cvx[:], d_cv[0:8, :])
            cvy = cst.tile([8, SC], F32)
            nc.sync.dma_start(cvy[:], d_cv[8:16, :])
            cf = cst.tile([SC, 17], F32)
            nc.sync.dma_start(cf[:], d_cf)
            Rm = cst.tile([8, 180], F32)
            nc.sync.dma_start(Rm[:], d_R)
            cfT = cst.tile([18, SC], F16)
            nc.sync.dma_start(cfT[:], d_cfT)
            R2c = cst.tile([18, NQ_C * 1024], F16)
            nc.sync.dma_start(R2c[:], d_R2c)
            I128 = cst.tile([SC, SC], F16)
            nc.sync.dma_start(I128[:], d_I128)
            bG = cst.tile([102, 2 * NBP], F16)
            I8r = cst.tile([8, 8], F32)
            nc.scalar.dma_start(I8r[:], d_I8)
            for j in range(4):
                nc.scalar.dma_start(bG[32 * j:32 * j + 6, :], d_bG)
            tb = cst.tile([SC, 512], BF16)
            thv = cst.tile([SC, 1], F32)
            nc.scalar.dma_start(thv[:], d_Th)
            b25 = cst.tile([SC, 1], F32)
            nc.vector.memset(b25[:], -25.0)
            nc.scalar.dma_start(tb[:], d_tb)
            Kv = cst.tile([SC, 1], F32)
            nc.scalar.dma_start(Kv[:], d_Kv)

            # ---- pts/vel/accel in [s, col] layout ----
            o1x = sml.tile([SC, 180], F32, tag="sm")
            nc.tensor.matmul(o1x[:], cvx[:], Rm[:], start=True, stop=True)
            o1y = sml.tile([SC, 180], F32, tag="sm")
            nc.tensor.matmul(o1y[:], cvy[:], Rm[:], start=True, stop=True)
            ox = cst.tile([SC, 180], F32)
            nc.vector.tensor_copy(ox[:], o1x[:])
            oy = cst.tile([SC, 180], F32)
            nc.vector.tensor_copy(oy[:], o1y[:])

            # ---- pts in [p, s] layout -> paug rows ----
            ptx = sml.tile([P, SC], F32, tag="sm")
            nc.tensor.matmul(ptx[:], Rm[:, 0:P], cvx[:], start=True, stop=True)
            pty = sml.tile([P, SC], F32, tag="sm")
            nc.tensor.matmul(pty[:], Rm[:, 0:P], cvy[:], start=True, stop=True)
            ptxs = cst.tile([P, SC], F16)
            nc.vector.tensor_copy(ptxs[:], ptx[:])
            ptys = cst.tile([P, SC], F16)
            nc.vector.tensor_copy(ptys[:], pty[:])

            pgi = paugp.tile([102, P * SC], F16)  # rows 32j..+5: [px; py; 1; 1; mhi_in; mlo_in]
            pgo = paugp.tile([102, P * SC], F16)  # rows 32j..+5: [px; py; 1; 1; mhi_out; mlo_out]
            qs = [nc.sync, nc.scalar, nc.gpsimd]
            di = 0
            for j in range(4):
                for pg in (pgi, pgo):
                    qs[di % 3].dma_start(pg[32 * j:32 * j + 1, :].rearrange("o (p s) -> o p s", p=P), ptxs[:]); di += 1
                    qs[di % 3].dma_start(pg[32 * j + 1:32 * j + 2, :].rearrange("o (p s) -> o p s", p=P), ptys[:]); di += 1
                    qs[di % 3].dma_start(pg[32 * j + 2:32 * j + 3, :], d_ones); di += 1
                    qs[di % 3].dma_start(pg[32 * j + 3:32 * j + 4, :], d_ones); di += 1

            # ---- boundary: per-quad pipeline, 4-way row/col tiled matmuls ----
            dTs = selcp.tile([SC, NQ * 64], F32)
            m2qs = [None] * NQ

            def em1(q):
                mraw = m2p.tile([SC, 16], F32, tag="m2f")   # cols 0:8 raw m, 8:16 scratch
                mq16 = m2p.tile([SC, 64], F16, tag="m2")    # 4x copies: mhi blk 0:32, mlo blk 32:64
                m2qs[q] = mq16
                # coarse center scores: [s, (p4, bd2, c128)] via K=18 matmul
                for hh in range(2):
                    cs = sml.tile([SC, 512], F32, tag="sm")
                    nc.tensor.matmul(cs[:], cfT[:],
                                     R2c[:, q * 1024 + hh * 512: q * 1024 + (hh + 1) * 512],
                                     start=True, stop=True)
                    nc.vector.tensor_reduce(
                        mraw[:, 4 * hh: 4 * hh + 4],
                        cs[:].rearrange("s (g c) -> s g c", c=NCC),
                        axis=AX.X, op=ALU.max)
                # fp16 hi/lo split: mhi16 = f16(m); mlo16 = f16(m - mhi16)
                nc.vector.tensor_copy(mq16[:, 0:8], mraw[:, 0:8])
                nc.vector.tensor_copy(mraw[:, 8:16], mq16[:, 0:8])
                nc.vector.tensor_sub(mraw[:, 8:16], mraw[:, 0:8], mraw[:, 8:16])
                nc.vector.tensor_copy(mq16[:, 32:40], mraw[:, 8:16])
                # replicate each 8-vec 3 more times within its 32-block
                nc.vector.tensor_copy(
                    mq16[:, 8:32].rearrange("s (r v) -> s r v", v=8),
                    mq16[:, 0:8].rearrange("s (r v) -> s r v", r=1).to_broadcast((SC, 3, 8)))
                nc.vector.tensor_copy(
                    mq16[:, 40:64].rearrange("s (r v) -> s r v", v=8),
                    mq16[:, 32:40].rearrange("s (r v) -> s r v", r=1).to_broadcast((SC, 3, 8)))
                mT = sml.tile([64, SC], F32, tag="sm")
                nc.tensor.matmul(mT[:], mq16[:], I128[:], start=True, stop=True)
                mTs = wk.tile([64, SC], F16, tag="mts")
                nc.scalar.copy(mTs[:], mT[:])
                qc = slice(q * 512, (q + 1) * 512)
                nc.sync.dma_start(
                    pgi[4:102:32, qc].rearrange("g (j2 s) -> g j2 s", j2=4), mTs[0:32:2, :])
                nc.scalar.dma_start(
                    pgo[4:102:32, qc].rearrange("g (j2 s) -> g j2 s", j2=4), mTs[1:32:2, :])
                nc.sync.dma_start(
                    pgi[5:102:32, qc].rearrange("g (j2 s) -> g j2 s", j2=4), mTs[32:64:2, :])
                nc.scalar.dma_start(
                    pgo[5:102:32, qc].rearrange("g (j2 s) -> g j2 s", j2=4), mTs[33:64:2, :])

            def em2(q):
                selc = selcp2.tile([8, 1024], F32, tag="selc")
                for bd in range(2):
                    pg = pgi if bd == 0 else pgo
                    sp = sml.tile([40, 512], F32, tag="sm")
                    for hw in range(4):  # windows of 2 chunks
                        t2 = big.tile([SC, NBP], F32, tag="big")
                        for cc in range(2):
                            c = 2 * hw + cc
                            g = 32 * (c % 4)
                            nc.tensor.matmul(
                                t2[:, cc * 512:(cc + 1) * 512],
                                bG[g:g + 6, bd * NBP + c * SC: bd * NBP + (c + 1) * SC],
                                pg[g:g + 6, q * 512:(q + 1) * 512],
                                start=True, stop=True, tile_position=(g, 0))
                        Ht = hbuf.tile([SC, NBP], BF16, tag="h")
                        if bd == 0:
                            nc.scalar.activation(Ht[:], t2[:], AF.Exp, scale=Kv[:], bias=b25[:])
                        else:
                            nc.vector.tensor_scalar(Ht[:], t2[:], thv[:], None, op0=ALU.is_le)
                        for cc in range(2):
                            c = 2 * hw + cc
                            cg = 32 * (c % 2)
                            nc.tensor.matmul(
                                sp[cg:cg + 8, :], tb[:, (bd * 8 + c) * 32:(bd * 8 + c) * 32 + 8],
                                Ht[:, cc * 512:(cc + 1) * 512],
                                start=(c < 2), stop=(c >= 6), tile_position=(0, cg))
                    nc.scalar.copy(selc[:, bd * 512:(bd + 1) * 512], sp[0:8, :])
                    nc.vector.tensor_add(selc[:, bd * 512:(bd + 1) * 512],
                                         selc[:, bd * 512:(bd + 1) * 512], sp[32:40, :])
                dTq = sml.tile([SC, 64], F32, tag="sm")
                for j4 in range(4):
                    for bd in range(2):
                        off = bd * 512 + j4 * SC
                        nc.tensor.matmul(
                            dTq[:, j4 * 16 + bd * 8: j4 * 16 + (bd + 1) * 8],
                            selc[:, off: off + SC], I8r[:],
                            start=True, stop=True)
                nc.scalar.copy(dTs[:, q * 64:(q + 1) * 64], dTq[:])

            for q in range(NQ + 3):
                if q < NQ:
                    em1(q)
                if q >= 3:
                    em2(q - 3)

            # ---- speeds / accel pipeline [128, 60] ----
            vx, vy, ax_, ay = (ox[:, 60:120], oy[:, 60:120], ox[:, 120:180], oy[:, 120:180])
            spd2 = wk.tile([SC, P], F32)
            nc.vector.tensor_mul(spd2[:], vx, vx)
            t0 = wk.tile([SC, P], F32)
            nc.vector.tensor_mul(t0[:], vy, vy)
            nc.vector.tensor_add(spd2[:], spd2[:], t0[:])
            spd = wk.tile([SC, P], F32)
            nc.scalar.activation(spd[:], spd2[:], AF.Sqrt)
            rspd = wk.tile([SC, P], F32)
            nc.vector.reciprocal(rspd[:], spd[:])
            adv = wk.tile([SC, P], F32)
            nc.vector.tensor_mul(adv[:], ax_, vx)
            nc.vector.tensor_mul(t0[:], ay, vy)
            nc.vector.tensor_add(adv[:], adv[:], t0[:])
            lin = wk.tile([SC, P], F32)
            nc.vector.tensor_mul(lin[:], adv[:], rspd[:])
            a2 = wk.tile([SC, P], F32)
            nc.vector.tensor_mul(a2[:], ax_, ax_)
            nc.vector.tensor_mul(t0[:], ay, ay)
            nc.vector.tensor_add(a2[:], a2[:], t0[:])
            nc.vector.tensor_mul(t0[:], lin[:], lin[:])
            nc.vector.tensor_sub(a2[:], a2[:], t0[:])  # ca^2 (may be ~-eps)
            camax2 = wk.tile([SC, 1], F32)
            nc.vector.tensor_reduce(camax2[:], a2[:], axis=AX.X, op=ALU.max)
            nc.vector.tensor_scalar_max(camax2[:], camax2[:], 0.0)
            camax = wk.tile([SC, 1], F32)
            nc.scalar.activation(camax[:], camax2[:], AF.Sqrt)

            avg = wk.tile([SC, 1], F32)
            nc.vector.tensor_reduce(avg[:], spd[:], axis=AX.X, op=ALU.add)

            # braking interp: bl = y0 + sum_i m_i * clip(spd - x_i, 0, dx_i)
            bl = wk.tile([SC, P], F32)
            nc.vector.memset(bl[:], float(y0))
            ti = wk.tile([SC, P], F32)
            for i in range(NSEG):
                nc.vector.tensor_scalar(ti[:], spd[:], float(interp_x[i]), 0.0, op0=ALU.subtract, op1=ALU.max)
                nc.vector.tensor_scalar(ti[:], ti[:], float(interp_dx[i]), float(interp_m[i]), op0=ALU.min, op1=ALU.mult)
                nc.vector.tensor_add(bl[:], bl[:], ti[:])
            bv = wk.tile([SC, P], F32)
            nc.vector.tensor_sub(bv[:], lin[:], bl[:])
            worst = wk.tile([SC, 1], F32)
            nc.vector.tensor_reduce(worst[:], bv[:], axis=AX.X, op=ALU.min)
            nc.vector.tensor_scalar_min(worst[:], worst[:], 0.0)

            # ---- phase C: dist + maxes (transposes done per-quad above) ----
            bmax = wk.tile([SC, 1], F32)
            for bd in range(2):
                Se = wk.tile([SC, P], F32, tag="c0")
                nc.vector.tensor_add(Se[:], dTs[:, bd * 8 + 0:960:16], dTs[:, bd * 8 + 1:960:16])
                Scx = wk.tile([SC, P], F32, tag="c1")
                nc.vector.tensor_add(Scx[:], dTs[:, bd * 8 + 2:960:16], dTs[:, bd * 8 + 3:960:16])
                Scy = wk.tile([SC, P], F32, tag="c2")
                nc.vector.tensor_add(Scy[:], dTs[:, bd * 8 + 4:960:16], dTs[:, bd * 8 + 5:960:16])
                Sn = dTs[:, bd * 8 + 6:960:16]
                Se = Se[:]
                Scx = Scx[:]
                Scy = Scy[:]
                n1 = wk.tile([SC, P], F32, tag="d1")
                nc.vector.tensor_mul(n1[:], ox[:, 0:P], Scx)
                n2 = wk.tile([SC, P], F32, tag="d2")
                nc.vector.tensor_mul(n2[:], oy[:, 0:P], Scy)
                nc.vector.tensor_sub(n1[:], Se, n1[:])
                nc.vector.tensor_sub(n1[:], n1[:], n2[:])
                rs = wk.tile([SC, P], F32, tag="d3")
                nc.vector.reciprocal(rs[:], Sn)
                nc.vector.tensor_mul(n1[:], n1[:], rs[:])
                dm = wk.tile([SC, 1], F32, tag="d4")
                nc.vector.tensor_reduce(dm[:], n1[:], axis=AX.X, op=ALU.max)
                if bd == 0:
                    nc.vector.tensor_copy(bmax[:], dm[:])
                else:
                    nc.vector.tensor_max(bmax[:], bmax[:], dm[:])
            nc.vector.tensor_scalar_max(bmax[:], bmax[:], 0.0)

            # ---- per-sample scores -> w ----
            args = wk.tile([SC, 1], F32)
            nc.vector.tensor_scalar(args[:], avg[:], float(BETA_SPEED / P), 0.0, op0=ALU.mult, op1=ALU.add)
            nc.vector.tensor_add(args[:], args[:], worst[:])
            ca_pen = wk.tile([SC, 1], F32)
            nc.vector.tensor_scalar(ca_pen[:], camax[:], float(MAX_CA), 0.0, op0=ALU.subtract, op1=ALU.max)
            nc.vector.tensor_sub(args[:], args[:], ca_pen[:])
            e1 = wk.tile([SC, 1], F32)
            nc.scalar.activation(e1[:], args[:], AF.Exp)
            e2 = wk.tile([SC, 1], F32)
            nc.scalar.activation(e2[:], bmax[:], AF.Exp, scale=-1.0)
            nc.vector.tensor_scalar_max(e2[:], e2[:], 1e-32)
            w = wk.tile([SC, 1], F32)
            nc.vector.tensor_mul(w[:], e1[:], e2[:])

            nc.sync.dma_start(d_diag[:, 0:1], w[:])

            # ---- partial sums ----
            op17 = sml.tile([17, 1], F32, tag="sm")
            nc.tensor.matmul(op17[:], cf[:], w[:], start=True, stop=True)
            o17 = wk.tile([17, 1], F32)
            nc.vector.tensor_copy(o17[:], op17[:])
            nc.sync.dma_start(d_out, o17[:])

    nc.compile()
    return nc


def _host_prep(curve, noise, deltaT, speeds_x, braking_y, bezierM, bezierMd, bezierM2d,
               inner_boundary, inner_normals, outer_boundary, outer_normals):
    f64 = np.float64
    dT = float(deltaT)
    curves = (curve[None].astype(f64) + noise.astype(f64))  # [1024, 8, 2]

    # R [8, 180]
    M = bezierM.astype(f64)
    Md = bezierMd.astype(f64)
    M2d = bezierM2d.astype(f64)
    D1 = _diff_mat(7)
    D1b = _diff_mat(6)[:, :7]
    R = np.zeros((8, 180), f64)
    R[:, 0:60] = M.T
    R[:, 60:120] = (7.0 / dT) * (Md @ D1).T
    R[:, 120:180] = (42.0 / (dT * dT)) * (M2d @ D1b @ D1).T

    # C-shift keeps all scores s1' = |p|^2 - d^2 - Csh strictly negative so
    # FP22 truncation of m (toward zero) can only raise it -> t2 >= 0 exact.
    cmax = max(float(np.abs(curves).max()), 1.0)
    Csh = 2.0 * cmax * cmax + 1.0

    def trunc22(x):
        x32 = np.asarray(x, np.float32).copy()
        u = x32.view(np.uint32)
        u &= np.uint32(0xFFFFF000)
        return x32.astype(f64)

    # boundary tables
    def btab(bpts, bnrm):
        b = bpts.astype(f64)
        n = bnrm.astype(f64)
        b2 = (b * b).sum(1)
        e = (b * n).sum(1)
        A = np.zeros((3, NBP), f64)
        A[0, :NB] = 2 * b[:, 0]
        A[1, :NB] = 2 * b[:, 1]
        A[2, :NB] = -(b2 + Csh)
        A[2, NB:] = -1e30
        G = np.zeros((6, NBP), f64)
        G[0, :NB] = -2 * b[:, 0]
        G[1, :NB] = -2 * b[:, 1]
        b2hi = np.float16(b2 + Csh).astype(f64)
        G[2, :NB] = b2hi
        G[3, :NB] = np.float16(b2 + Csh - b2hi).astype(f64)
        G[2, NB:] = 60000.0
        G[4, :] = 1.0
        G[5, :] = 1.0
        T = np.zeros((NBP, 4), f64)
        T[:NB, 0] = e
        T[:NB, 1] = n[:, 0]
        T[:NB, 2] = n[:, 1]
        T[:NB, 3] = 1.0
        return A, G, T, b2.max()

    Ai, Gi, Ti, m2i = btab(inner_boundary, inner_normals)
    Ao, Go, To, m2o = btab(outer_boundary, outer_normals)
    bG = np.concatenate([Gi, Go], 1)

    # select table -> bf16 hi/lo pairs [ehi,elo,nxhi,nxlo,nyhi,nylo,1,0]
    def bf16_rne(x):
        x32 = np.asarray(x, np.float32)
        u = x32.view(np.uint32)
        r = ((u + 0x7FFF + ((u >> 16) & 1)) & 0xFFFF0000).astype(np.uint32)
        return r.view(np.float32).astype(f64)

    tbl = np.concatenate([Ti, To], 0)  # [2048, 4] (e, nx, ny, 1)
    tbl8 = np.zeros((2048, 32), f64)
    for v in range(3):
        hi = bf16_rne(tbl[:, v])
        lo = bf16_rne(tbl[:, v] - hi)
        tbl8[:, 2 * v] = hi
        tbl8[:, 2 * v + 1] = lo
    tbl8[:, 6] = tbl[:, 3]  # the count/ones column
    tb_sb = np.ascontiguousarray(
        tbl8.reshape(2, 8, 128, 32).transpose(2, 0, 1, 3).reshape(128, 512))

    Bmax2 = max(m2i, m2o, 1.0)
    smax = 2.0 * cmax * np.sqrt(Bmax2) + Bmax2 + Csh + 2.0 * cmax * cmax

    # ---- coarse centers (farthest-point sampling) + center-score table ----
    def fps(pts, k):
        d = ((pts - pts[0]) ** 2).sum(1)
        idx = [0]
        for _ in range(k - 1):
            i = int(d.argmax())
            idx.append(i)
            d = np.minimum(d, ((pts - pts[i]) ** 2).sum(1))
        return np.array(idx)

    def kmedoid(pts, k):
        idx = fps(pts, k)
        C = pts[idx]
        for _ in range(5):
            d2 = ((pts[:, None, :] - C[None]) ** 2).sum(-1)
            a = d2.argmin(1)
            for j in range(k):
                msk = a == j
                if msk.any():
                    C[j] = pts[msk].mean(0)
        # snap to nearest actual boundary point (keeps m_hat <= true max)
        d2 = ((pts[:, None, :] - C[None]) ** 2).sum(-1)
        return d2.argmin(0)

    bi = inner_boundary.astype(f64)
    bo = outer_boundary.astype(f64)
    ci_idx = kmedoid(bi, NCC)
    co_idx = kmedoid(bo, NCC)

    # R2c [18, 15*1024]: cols (j4 in 4, bd in 2, c in 128); score = 2c.p - (|c|^2+Csh)
    cents = [bi[ci_idx], bo[co_idx]]  # each [128, 2]
    R2c = np.zeros((18, NQ_C * 1024), f64)
    for q in range(NQ_C):
        for j4 in range(4):
            p = 4 * q + j4
            for bd in range(2):
                cc = cents[bd]  # [128, 2]
                base = q * 1024 + j4 * 256 + bd * NCC
                c2C = (cc ** 2).sum(1) + Csh
                c2hi = np.float16(c2C).astype(f64)
                R2c[0:8, base:base + NCC] = np.outer(M[p, :], 2.0 * cc[:, 0])
                R2c[8:16, base:base + NCC] = np.outer(M[p, :], 2.0 * cc[:, 1])
                R2c[16, base:base + NCC] = -c2hi
                R2c[17, base:base + NCC] = -np.float16(c2C - c2hi).astype(f64)

    # ---- adaptive K from a coverage-gap bound (grid over the query region) ----
    qm = np.sqrt(2.0) * cmax + 0.5
    gs = np.linspace(-qm, qm, 161)
    Q = np.stack(np.meshgrid(gs, gs), -1).reshape(-1, 2)

    def gapbound(b, cidx):
        gap = 0.0
        for lo in range(0, len(Q), 4096):
            d2 = ((Q[lo:lo + 4096, None, :] - b[None]) ** 2).sum(-1)
            gap = max(gap, float((d2[:, cidx].min(1) - d2.min(1)).max()))
        return gap

    gb = 2.0 * max(gapbound(bi, ci_idx), gapbound(bo, co_idx)) + 0.3
    noise = 0.55 * max(smax / 3700.0, 0.05)
    K = float(min(2.0 ** 17 / smax, 60.0 / (gb + noise)))
    theta = float(noise + 3.0 / K)

    # interp constants
    xs = speeds_x.astype(f64)
    ys = braking_y.astype(f64)
    dx = np.diff(xs)
    dx_safe = np.where(dx > 0, dx, 1.0)
    m = np.where(dx > 0, np.diff(ys) / dx_safe, 0.0)

    # per-core shards
    import ml_dtypes
    tb_bf16 = tb_sb.astype(ml_dtypes.bfloat16)
    ins = []
    for c in range(NCORES):
        cs = curves[c * SC:(c + 1) * SC]  # [128, 8, 2]
        cv = np.ascontiguousarray(cs.transpose(2, 1, 0).reshape(16, SC)).astype(np.float32)
        cf17 = np.concatenate([cs.reshape(SC, 16), np.ones((SC, 1))], 1).astype(np.float32)
        cfTc = np.concatenate([cs[:, :, 0].T, cs[:, :, 1].T, np.ones((2, SC))], 0).astype(np.float16)
        ins.append(dict(
            cv=cv, cf17=cf17, cfT=cfTc,
            Rm=R.astype(np.float32), bG=bG.astype(np.float16),
            tb=tb_bf16, R2c=R2c.astype(np.float16),
            Th=np.full((SC, 1), theta, np.float32),
            I8=np.eye(8, dtype=np.float32), I128=np.eye(128, dtype=np.float16),
            Kv=np.full((SC, 1), -K, np.float32),
            ones_row=np.ones((1, SC * P), np.float16),
        ))
    return ins, (xs, dx_safe, m, float(ys[0]), K)


def kernel(curve, noise, deltaT, speeds_x, braking_y, bezierM, bezierMd, bezierM2d,
           inner_boundary, inner_normals, outer_boundary, outer_normals):
    in_maps, (xs, dxs, ms, y0, K) = _host_prep(
        curve, noise, deltaT, speeds_x, braking_y, bezierM, bezierMd, bezierM2d,
        inner_boundary, inner_normals, outer_boundary, outer_normals)

    key = (tuple(np.round(xs, 9)), tuple(np.round(ms, 9)), round(y0, 9))
    if key not in _cache:
        _cache.clear()
        _cache[key] = _build_program(xs, dxs, ms, y0)
    nc = _cache[key]

    res = bass_utils.run_bass_kernel_spmd(nc, in_maps, core_ids=list(range(NCORES)))
    outs = res.results
    num = np.zeros(16, np.float64)
    Z = 0.0
    for c in range(NCORES):
        o = np.asarray(outs[c]["out17"]).reshape(17)
        num += o[:16].astype(np.float64)
        Z += float(o[16])
    return (num / Z).reshape(8, 2).astype(np.float32)


if __name__ == "__main__":
    import reference
    inp = {k: np.asarray(v) for k, v in reference.setup_inputs().items()}
    out = kernel(**inp)
    exp = np.asarray(reference.reference(**reference.setup_inputs()))
    err = np.abs(out - exp).max() / (np.abs(exp).max() + 1e-12)
    print("Relative error:", err)

